# revision 11
# baseline (speedup 1.0000x reference)
"""DeepGCN (3-branch graph-attention GCN, 2 layers) on 8 Trainium2 NeuronCores.

Strategy (row-shard, transposed dataflow):
  - Node rows are sharded across 8 cores (512 rows each). The host feeds each
    core its row-shard of every adjacency ALREADY TRANSPOSED (aT = adj[rows].T,
    shape [4096, 512], contiguous) so all device DMA is dense row-major and the
    attention matrix is produced directly in "source-node on partitions"
    orientation -- the orientation the second matmul (attn @ h) needs. No
    on-chip transposes of any NxN matrix.
  - exp(sim/tau + log(adj+eps)) == adj * exp(sim/tau) (eps dropped; see below),
    so the log/mask path is a single elementwise multiply.
  - Softmax normalization is applied to the [H, 512] conv output (divide by
    row-sums obtained with a ones-vector matmul), never to the NxN matrix.
  - Cosine normalization: h is left unnormalized; 1/max(||h_i||,eps) is folded
    into the exp's per-partition scale (row factor) and into the 512 local
    columns of the moving operand (column factor).
  - Between layers, a 256KB-per-core AllGather rebuilds the full [128, 4096]
    h0-combined tensor on every core.
  - Big matmuls run as float32r (full PE rate; ~19-bit mantissa).

Numerics vs the reference:
  - dropping +eps inside the adjacency mask changes attention weights by
    O(1e-8 / rowsum) ~ 1e-11 relative: negligible.
  - softmax computed without max-subtraction: logits <= 1/tau + log(1+eps), so
    exp() cannot overflow; identical result up to fp rounding.
"""

import sys

sys.path.insert(0, "/opt/trn_rl_repo")

import numpy as np

N, F, H, C = 4096, 512, 128, 16
NCORES = 8
S = N // NCORES  # 512 local rows per core
JC = N // 128  # 32 chunks of source nodes
NK = N // 512  # 8 column chunks for 512-wide matmul streaming
FC = F // 128  # 4 feature chunks
EPS = 1e-8
TAU = 1.0

_CACHE = {}


def _build_nc(mock_gather=False):
    import concourse.bass as bass
    import concourse.bacc as bacc
    import concourse.tile as tile
    import concourse.mybir as mybir

    dt = mybir.dt
    f32 = dt.float32
    f32r = dt.float32r
    AF = mybir.ActivationFunctionType
    ALU = mybir.AluOpType

    nc = bacc.Bacc("TRN2", target_bir_lowering=False, debug=False, num_devices=NCORES)

    # ---- I/O ----
    xT_d = nc.dram_tensor("xT", [F, N], f32, kind="ExternalInput")
    aT_d = [nc.dram_tensor(f"aT{a}", [N, S], f32, kind="ExternalInput") for a in range(3)]
    w0_d = [nc.dram_tensor(f"w0_{a}", [F, H], f32, kind="ExternalInput") for a in range(3)]
    w1_d = [nc.dram_tensor(f"w1_{a}", [H, C], f32, kind="ExternalInput") for a in range(3)]
    wb0_d = nc.dram_tensor("wb0", [H, 3], f32, kind="ExternalInput")  # gate_a * bias0_a
    wb1_d = nc.dram_tensor("wb1", [C, 3], f32, kind="ExternalInput")  # gate_a * bias1_a
    gw0_d = nc.dram_tensor("gw0", [1, 3], f32, kind="ExternalInput")  # softmax(g0)
    gw1_d = nc.dram_tensor("gw1", [1, 3], f32, kind="ExternalInput")  # softmax(g1)
    id_d = nc.dram_tensor("ident", [128, 128], f32, kind="ExternalInput")
    ones_d = nc.dram_tensor("ones", [128, 1], f32, kind="ExternalInput")
    out_d = nc.dram_tensor("out", [S, C], f32, kind="ExternalOutput")

    def r(ap):
        return ap.bitcast(f32r)

    with tile.TileContext(nc) as tc:
        with (
            tc.tile_pool(name="const", bufs=1) as const,
            tc.tile_pool(name="persist", bufs=1) as persist,
            tc.tile_pool(name="stream", bufs=3) as stream,
            tc.tile_pool(name="small", bufs=2) as small,
            tc.tile_pool(name="dram", bufs=1, space="DRAM") as dram,
            tc.tile_pool(name="ps_sim", bufs=4, space="PSUM") as ps_simp,
            tc.tile_pool(name="ps_out", bufs=1, space="PSUM") as ps_outp,
            tc.tile_pool(name="ps_t", bufs=2, space="PSUM") as ps_tp,
            tc.tile_pool(name="ps_misc", bufs=1, space="PSUM") as ps_miscp,
        ):
            # ---- constants ----
            ident = const.tile([128, 128], f32, tag="ident")
            nc.sync.dma_start(ident, id_d[:, :])
            ones_k = const.tile([128, 1], f32r, tag="ones_k")  # K-dim ones (col sums)
            nc.sync.dma_start(ones_k, r(ones_d[:, :]))
            ones_m = const.tile([1, 128], f32, tag="ones_m")  # M-dim ones (broadcast)
            nc.vector.memset(ones_m, 1.0)
            wb0 = const.tile([H, 3], f32, tag="wb0")
            nc.sync.dma_start(wb0, wb0_d[:, :])
            wb1 = const.tile([C, 3], f32, tag="wb1")
            nc.sync.dma_start(wb1, wb1_d[:, :])
            gw0 = const.tile([1, 3], f32, tag="gw0")
            nc.sync.dma_start(gw0, gw0_d[:, :])
            gw1 = const.tile([1, 3], f32, tag="gw1")
            nc.sync.dma_start(gw1, gw1_d[:, :])
            w1t = []
            for a in range(3):
                t = const.tile([H, C], f32r, tag=f"w1t{a}")
                nc.sync.dma_start(t, r(w1_d[a][:, :]))
                w1t.append(t)
            w0t = []
            for a in range(3):
                t = const.tile([128, FC * H], f32r, tag=f"w0t{a}")
                for fc in range(FC):
                    nc.sync.dma_start(
                        t[:, fc * H:(fc + 1) * H],
                        r(w0_d[a][fc * 128:(fc + 1) * 128, :]),
                    )
                w0t.append(t)

            pid = nc.vector.partition_id()

            acc0 = persist.tile([H, S], f32, tag="acc0")  # layer-0 combined (T layout)
            h0cT = persist.tile([H, N], f32r, tag="h0cT")  # gathered full layer-0 out

            def adj_main(a, hT, hnat, invp, hnloc, nat_w, gw, wb, acc, layer):
                """Shared attention main loop + epilogue for one adjacency.

                hT: [*, N] features transposed; hnat: [128, JC*nat_w] natural;
                invp: [128, JC] inverse norms; hnloc: [*, S] normalized local
                columns; nat_w: H (layer 0) or C (layer 1); acc: accumulator.
                """
                S1 = small.tile([128, S], f32r, tag=f"S1_{layer}")
                ps_out = ps_outp.tile([nat_w, S], f32, tag="ps_out")
                for jb in range(JC // 2):
                    at2 = stream.tile([128, 2, 512], f32, tag="at2")
                    nc.sync.dma_start(
                        at2,
                        aT_d[a][jb * 256:(jb + 1) * 256, :].rearrange(
                            "(q p) f -> p q f", p=128
                        ),
                    )
                    for q in range(2):
                        jc = jb * 2 + q
                        ps_sim = ps_simp.tile([128, 512], f32, tag="ps_sim")
                        nc.tensor.matmul(
                            ps_sim,
                            r(hT[:, jc * 128:(jc + 1) * 128]),
                            r(hnloc),
                            start=True, stop=True,
                        )
                        e = stream.tile([128, 512], f32r, tag="e")
                        nc.scalar.activation(
                            e, ps_sim, AF.Exp, scale=invp[:, jc:jc + 1]
                        )
                        nc.vector.tensor_tensor(e, e.bitcast(f32), at2[:, q, :], ALU.mult)
                        if jc == 0:
                            nc.vector.tensor_copy(S1, e.bitcast(f32))
                        else:
                            nc.vector.tensor_tensor(S1, S1.bitcast(f32), e.bitcast(f32), ALU.add)
                        nc.tensor.matmul(
                            ps_out,
                            r(hnat[:, jc * nat_w:(jc + 1) * nat_w]), r(e),
                            start=(jc == 0), stop=(jc == JC - 1),
                        )
                # epilogue: normalize rows, gate-weight, bias, (relu,) combine
                ps_s = ps_miscp.tile([1, 512], f32, tag="ps_misc")
                nc.tensor.matmul(ps_s, r(ones_k), r(S1), start=True, stop=True)
                invs = small.tile([1, S], f32, tag=f"invs_{layer}")
                nc.vector.reciprocal(invs, ps_s)
                nc.vector.tensor_scalar_mul(invs, invs, gw[0:1, a:a + 1])
                ps_b2 = ps_miscp.tile([nat_w, 512], f32, tag="ps_misc")
                nc.tensor.matmul(
                    ps_b2, ones_m[0:1, 0:nat_w], invs, start=True, stop=True
                )
                t0 = small.tile([nat_w, S], f32, tag=f"t0_{layer}")
                nc.vector.tensor_copy(t0, ps_b2)
                nc.vector.tensor_tensor(t0, ps_out, t0, ALU.mult)
                if layer == 0:
                    if a == 0:
                        nc.vector.tensor_scalar(
                            acc, t0, wb[:, a:a + 1], 0.0, ALU.add, ALU.max
                        )
                    else:
                        r0 = small.tile([nat_w, S], f32, tag="r0")
                        nc.vector.tensor_scalar(
                            r0, t0, wb[:, a:a + 1], 0.0, ALU.add, ALU.max
                        )
                        nc.vector.tensor_tensor(acc, acc, r0, ALU.add)
                else:
                    if a == 0:
                        nc.vector.tensor_scalar(
                            acc, t0, wb[:, a:a + 1], None, ALU.add
                        )
                    else:
                        nc.vector.scalar_tensor_tensor(
                            acc, t0, wb[:, a:a + 1], acc, ALU.add, ALU.add
                        )

            def inv_norms(pool, hnat, nat_w, layer, a):
                """[128, JC] inverse clamped row norms from natural-layout h."""
                nrm2 = small.tile([128, JC], f32, tag=f"nrm2_{layer}")
                for jc in range(JC):
                    scr = stream.tile([128, nat_w], f32, tag=f"sq_scr{layer}")
                    nc.scalar.activation(
                        scr, hnat[:, jc * nat_w:(jc + 1) * nat_w].bitcast(f32), AF.Square,
                        accum_out=nrm2[:, jc:jc + 1],
                    )
                nc.scalar.sqrt(nrm2, nrm2)
                nc.vector.tensor_scalar_max(nrm2, nrm2, EPS)
                inv = pool.tile([128, JC], f32, tag=f"invp_{layer}", bufs=2)
                nc.vector.reciprocal(inv, nrm2)
                return inv

            def local_cols(pool, hT, inv, nat_w, layer):
                """hnloc[*, S]: local columns of hT scaled by their inv norm."""
                invL = small.tile([128, 4], f32, tag=f"invL_{layer}")
                nc.vector.tensor_copy(invL, inv[:, bass.ts(pid, 4)])
                pslt = ps_tp.tile([1, 512], f32, tag="ps_t")
                for q in range(4):
                    nc.tensor.transpose(
                        pslt[0:1, q * 128:(q + 1) * 128], invL[:, q:q + 1], ident
                    )
                invLT = small.tile([1, 512], f32, tag=f"invLT_{layer}")
                nc.vector.tensor_copy(invLT, pslt)
                psb = ps_miscp.tile([128, 512], f32, tag="ps_misc")
                for q in range(4):
                    nc.tensor.matmul(
                        psb[0:nat_w, q * 128:(q + 1) * 128], ones_m[0:1, 0:nat_w],
                        invLT[0:1, q * 128:(q + 1) * 128],
                        start=True, stop=True,
                    )
                hl = pool.tile([nat_w, S], f32r, tag=f"hnloc_{layer}", bufs=2)
                nc.vector.tensor_tensor(
                    hl, hT[:, bass.ts(pid, S)].bitcast(f32), psb[0:nat_w, :], ALU.mult
                )
                return hl

            # ================= LAYER 0 =================
            with tc.tile_pool(name="l0", bufs=1) as l0:
                # h0T_a = w0_a.T @ x.T for all three branches, streaming xT once
                h0T = [
                    l0.tile([H, N], f32r, tag=f"h0T{a}", name=f"h0T{a}")
                    for a in range(3)
                ]
                for nk in range(NK):
                    ps3 = [
                        ps_simp.tile([128, 512], f32, tag="ps_sim", name=f"ps3_{i}")
                        for i in range(3)
                    ]
                    for fc in range(FC):
                        xc = stream.tile([128, 512], f32r, tag="xc")
                        nc.sync.dma_start(
                            xc,
                            r(xT_d[fc * 128:(fc + 1) * 128, nk * 512:(nk + 1) * 512]),
                        )
                        for a in range(3):
                            nc.tensor.matmul(
                                ps3[a],
                                r(w0t[a][:, fc * H:(fc + 1) * H]),
                                r(xc),
                                start=(fc == 0), stop=(fc == FC - 1),
                            )
                    for a in range(3):
                        nc.scalar.copy(h0T[a][:, nk * 512:(nk + 1) * 512], ps3[a])

                for a in range(3):
                    # natural layout via PE transposes
                    hN = l0.tile([128, N], f32r, tag="h0nat", bufs=2)
                    for jc in range(JC):
                        pst = ps_tp.tile([128, 128], f32, tag="ps_t")
                        nc.tensor.transpose(
                            pst, h0T[a][:, jc * 128:(jc + 1) * 128].bitcast(f32), ident
                        )
                        nc.vector.tensor_copy(hN[:, jc * 128:(jc + 1) * 128], pst)
                    inv = inv_norms(l0, hN, H, 0, a)
                    hl = local_cols(l0, h0T[a], inv, H, 0)
                    adj_main(a, h0T[a], hN, inv, hl, H, gw0, wb0, acc0, 0)

            # ================= all-gather =================
            cc_in = dram.tile([H, S], f32, tag="cc_in")
            cc_out = dram.tile([NCORES * H, S], f32, tag="cc_out")
            nc.sync.dma_start(cc_in, acc0)
            if mock_gather:
                for b in range(NCORES):
                    nc.sync.dma_start(
                        h0cT[:, b * S:(b + 1) * S], r(cc_in[:, :])
                    )
            else:
                nc.gpsimd.collective_compute(
                    "AllGather",
                    mybir.AluOpType.bypass,
                    replica_groups=[list(range(NCORES))],
                    ins=[cc_in[:].opt()],
                    outs=[cc_out[:].opt()],
                )
                for b in range(NCORES):
                    nc.sync.dma_start(
                        h0cT[:, b * S:(b + 1) * S], r(cc_out[b * H:(b + 1) * H, :])
                    )

            # ================= LAYER 1 =================
            with tc.tile_pool(name="l1", bufs=1) as l1:
                acc1 = l1.tile([C, S], f32, tag="acc1")
                for a in range(3):
                    hT = l1.tile([C, N], f32r, tag="h1T", bufs=2)
                    for nk in range(NK):
                        ps1 = ps_miscp.tile([C, 512], f32, tag="ps_misc")
                        nc.tensor.matmul(
                            ps1, r(w1t[a]), r(h0cT[:, nk * 512:(nk + 1) * 512]),
                            start=True, stop=True,
                        )
                        nc.vector.tensor_copy(hT[:, nk * 512:(nk + 1) * 512], ps1)

                    hN = l1.tile([128, JC * C], f32r, tag="h1nat", bufs=2)
                    for jc in range(JC):
                        ps1n = ps_tp.tile([128, C], f32, tag="ps_t")
                        nc.tensor.matmul(
                            ps1n, r(h0cT[:, jc * 128:(jc + 1) * 128]), r(w1t[a]),
                            start=True, stop=True,
                        )
                        nc.vector.tensor_copy(hN[:, jc * C:(jc + 1) * C], ps1n)
                    inv = inv_norms(l1, hN, C, 1, a)
                    hl = local_cols(l1, hT, inv, C, 1)
                    adj_main(a, hT, hN, inv, hl, C, gw1, wb1, acc1, 1)

                # ---- log_softmax + output ----
                for q in range(4):
                    pst = ps_tp.tile([128, C], f32, tag="ps_t")
                    nc.tensor.transpose(
                        pst, acc1[:, q * 128:(q + 1) * 128], ident[0:C, 0:C]
                    )
                    negm = small.tile([128, 1], f32, tag="negm")
                    nc.vector.tensor_reduce(
                        negm, pst, mybir.AxisListType.X, ALU.max, negate=True
                    )
                    xs = small.tile([128, C], f32, tag="xs")
                    nc.vector.tensor_scalar_add(xs, pst, negm)
                    e2 = small.tile([128, C], f32, tag="e2")
                    se = small.tile([128, 1], f32, tag="se")
                    nc.scalar.activation(e2, xs, AF.Exp, accum_out=se)
                    lse = small.tile([128, 1], f32, tag="lse")
                    nc.scalar.activation(lse, se, AF.Ln)
                    res = small.tile([128, C], f32, tag="res")
                    nc.vector.tensor_scalar(res, xs, lse, None, ALU.subtract)
                    nc.sync.dma_start(out_d[q * 128:(q + 1) * 128, :], res)

    nc.compile()
    return nc


def _get_nc():
    if "nc" not in _CACHE:
        _CACHE["nc"] = _build_nc()
    return _CACHE["nc"]


def _softmax_np(v):
    v = np.asarray(v, np.float64)
    e = np.exp(v - v.max())
    return (e / e.sum()).astype(np.float32)


def make_in_maps(x, adj_low, adj_high, adj_mid,
                 w_low0, b_low0, w_high0, b_high0, w_mid0, b_mid0,
                 w_low1, b_low1, w_high1, b_high1, w_mid1, b_mid1,
                 g0, g1):
    adjs = [np.asarray(a, np.float32) for a in (adj_low, adj_high, adj_mid)]
    w0s = [np.ascontiguousarray(np.asarray(w, np.float32))
           for w in (w_low0, w_high0, w_mid0)]
    w1s = [np.ascontiguousarray(np.asarray(w, np.float32))
           for w in (w_low1, w_high1, w_mid1)]
    b0s = [np.asarray(b, np.float32) for b in (b_low0, b_high0, b_mid0)]
    b1s = [np.asarray(b, np.float32) for b in (b_low1, b_high1, b_mid1)]
    gw0 = _softmax_np(g0)
    gw1 = _softmax_np(g1)
    xT = np.ascontiguousarray(np.asarray(x, np.float32).T)
    wb0 = np.ascontiguousarray(np.stack([gw0[a] * b0s[a] for a in range(3)], axis=1))
    wb1 = np.ascontiguousarray(np.stack([gw1[a] * b1s[a] for a in range(3)], axis=1))
    ident = np.eye(128, dtype=np.float32)

    common = {
        "xT": xT,
        "wb0": wb0, "wb1": wb1,
        "gw0": gw0.reshape(1, 3), "gw1": gw1.reshape(1, 3),
        "ident": ident,
        "ones": np.ones((128, 1), dtype=np.float32),
    }
    for a in range(3):
        common[f"w0_{a}"] = w0s[a]
        common[f"w1_{a}"] = w1s[a]

    in_maps = []
    for c in range(NCORES):
        m = dict(common)
        for a, adj in enumerate(adjs):
            m[f"aT{a}"] = np.ascontiguousarray(adj[c * S:(c + 1) * S, :].T)
        in_maps.append(m)
    return in_maps


def kernel(**inputs) -> np.ndarray:
    from concourse import bass_utils

    nc = _get_nc()
    in_maps = make_in_maps(**inputs)
    res = bass_utils.run_bass_kernel_spmd(
        nc, in_maps, core_ids=list(range(NCORES))
    )
    return np.concatenate([res.results[c]["out"] for c in range(NCORES)], axis=0)
